# revision 5
# baseline (speedup 1.0000x reference)
"""2D DWT (db2, FFT-equivalent circular conv) as TensorE matmuls on 8 trn2 cores.

Math: for each (b,c) slice X (128x128), with F[k,j] = w[t] at k=(2j+2-t) mod 128
(the circular 4-tap filter + stride-2 decimation as a 128x64 matrix):
    LL = Fl^T X Fl,  LH = Fh^T X Fl,  HL = Fl^T X Fh,  HH = Fh^T X Fh.
With W2 = [Fl | Fh] (128x128):
    stage 1:  out1 = X^T @ W2 = [B_lT | B_hT]           (w on partitions)
    stage 2:  out2 = W2^T @ out1 = [[LL^T, LH^T], [HL^T, HH^T]]
out2 has partitions = j (W-direction output), free = i (H-direction output);
the final transpose of each 64x64 quadrant happens on the host at gather time.

Stage 1 runs as three fp16 matmuls accumulating in PSUM (X and W2 split into
fp16 hi+lo parts on the host; the dropped lo*lo term is ~2^-22 relative, so
accuracy stays at fp32 level) — fp16 streams at 1 cycle/col vs fp32's 4.
X is pre-scaled by 4096 on the host so near-zero values stay out of fp16
subnormal range; the scale is divided out in the PSUM->SBUF copy.
Stage 2 stays fp32: its moving operand is produced on-chip, and any exact
split there would cost as much as the fp32 matmul itself.

Sharding: 768 (b,c) slices split contiguously, 96 per core; pure data parallel.
Per-core input shards are transposed on the host to (h, s, w) so every DMA
reads multi-KB contiguous runs per partition.
"""

import numpy as np

_NCORES = 8
_S = 96          # slices per core
_G = 16          # max slices per chunk
_N = 128
_SCALE = 4096.0  # fp16 subnormal guard; divided out in stage-1 copies

_compiled = None


def _build_w2(w_l: np.ndarray, w_h: np.ndarray) -> np.ndarray:
    W2 = np.zeros((_N, _N), dtype=np.float32)
    for col, w in ((0, w_l), (64, w_h)):
        w = np.asarray(w, dtype=np.float32).reshape(-1)
        L = w.shape[0]
        for j in range(_N // 2):
            for t in range(L):
                W2[(2 * j + L // 2 - t) % _N, col + j] += w[t]
    return W2


def _build_nc():
    import concourse.bacc as bacc
    import concourse.tile as tile
    import concourse.mybir as mybir

    f32 = mybir.dt.float32
    f16 = mybir.dt.float16
    nc = bacc.Bacc("TRN2", target_bir_lowering=False, debug=False)

    xh = nc.dram_tensor("xh", [_N, _S, _N], f16, kind="ExternalInput")  # (h, s, w)
    xl = nc.dram_tensor("xl", [_N, _S, _N], f16, kind="ExternalInput")
    w2h = nc.dram_tensor("w2h", [_N, _N], f16, kind="ExternalInput")
    w2l = nc.dram_tensor("w2l", [_N, _N], f16, kind="ExternalInput")
    w2 = nc.dram_tensor("w2", [_N, _N], f32, kind="ExternalInput")
    out_t = nc.dram_tensor("out_t", [_N, _S, _N], f32, kind="ExternalOutput")

    inv = 1.0 / _SCALE
    # graduated chunks: small at start (PE starts fast) and end (short tail)
    chunks = [2, 2, 4, 8, 16, 16, 16, 16, 8, 4, 2, 2]
    assert sum(chunks) == _S
    with tile.TileContext(nc) as tc:
        with (
            tc.tile_pool(name="singles", bufs=1) as singles,
            tc.tile_pool(name="xinh", bufs=3) as xinh,
            tc.tile_pool(name="xinl", bufs=3) as xinl,
            tc.tile_pool(name="mid", bufs=2) as mid,
            tc.tile_pool(name="out", bufs=2) as outp,
            tc.tile_pool(name="ps1", bufs=4, space="PSUM") as ps1p,
            tc.tile_pool(name="ps2", bufs=2, space="PSUM") as ps2p,
        ):
            w2_sb = singles.tile([_N, _N], f32)
            w2h_sb = singles.tile([_N, _N], f16)
            w2l_sb = singles.tile([_N, _N], f16)
            nc.sync.dma_start(out=w2_sb[:], in_=w2[:])
            nc.sync.dma_start(out=w2h_sb[:], in_=w2h[:])
            nc.sync.dma_start(out=w2l_sb[:], in_=w2l[:])

            nv = ns = 0  # vector/scalar copy round-robin counters
            c0 = 0
            for G in chunks:
                xh_sb = xinh.tile([_N, _G * _N], f16, tag="xh")
                xl_sb = xinl.tile([_N, _G * _N], f16, tag="xl")
                nc.sync.dma_start(
                    out=xh_sb[:, : G * _N].rearrange("p (s w) -> p s w", s=G),
                    in_=xh[:, c0 : c0 + G, :],
                )
                nc.sync.dma_start(
                    out=xl_sb[:, : G * _N].rearrange("p (s w) -> p s w", s=G),
                    in_=xl[:, c0 : c0 + G, :],
                )
                out1_sb = mid.tile([_N, _G * _N], f32, tag="mid")
                for s in range(G):
                    ps1 = ps1p.tile([_N, _N], f32)
                    lh = xh_sb[:, s * _N : (s + 1) * _N]
                    ll = xl_sb[:, s * _N : (s + 1) * _N]
                    nc.tensor.matmul(ps1[:], lhsT=lh, rhs=w2h_sb[:], start=True, stop=False)
                    nc.tensor.matmul(ps1[:], lhsT=lh, rhs=w2l_sb[:], start=False, stop=False)
                    nc.tensor.matmul(ps1[:], lhsT=ll, rhs=w2h_sb[:], start=False, stop=True)
                    dst = out1_sb[:, s * _N : (s + 1) * _N]
                    if (nv + ns) % 2 == 0:
                        nc.vector.tensor_scalar_mul(dst, ps1[:], inv)
                        nv += 1
                    else:
                        nc.scalar.activation(
                            dst, ps1[:], mybir.ActivationFunctionType.Copy, scale=inv
                        )
                        ns += 1

                out2_sb = outp.tile([_N, _G * _N], f32, tag="out")
                ncols = G * _N
                g0 = 0
                while g0 < ncols:
                    gw = min(512, ncols - g0)
                    ps2 = ps2p.tile([_N, 512], f32)
                    nc.tensor.matmul(
                        ps2[:, :gw],
                        lhsT=w2_sb[:],
                        rhs=out1_sb[:, g0 : g0 + gw],
                        start=True,
                        stop=True,
                    )
                    dst = out2_sb[:, g0 : g0 + gw]
                    if (nv + ns) % 2 == 0:
                        nc.vector.tensor_copy(out=dst, in_=ps2[:, :gw])
                        nv += 1
                    else:
                        nc.scalar.copy(out=dst, in_=ps2[:, :gw])
                        ns += 1
                    g0 += gw

                nc.sync.dma_start(
                    out=out_t[:, c0 : c0 + G, :],
                    in_=out2_sb[:, : G * _N].rearrange("p (s f) -> p s f", s=G),
                )
                c0 += G
    nc.finalize()
    return nc


def _get_compiled():
    global _compiled
    if _compiled is None:
        _compiled = _build_nc()
    return _compiled


def run_on_hw(x: np.ndarray, w_l: np.ndarray, w_h: np.ndarray, trace: bool = False):
    """Returns ((LL, LH, HL, HH), exec_time_ns or None)."""
    from concourse.bass_utils import run_bass_kernel_spmd

    x = np.asarray(x, dtype=np.float32)
    W2 = _build_w2(np.asarray(w_l), np.asarray(w_h))
    W2h = W2.astype(np.float16)
    W2l = (W2 - W2h.astype(np.float32)).astype(np.float16)

    xf = x.reshape(-1, _N, _N)  # (768, 128, 128)
    nc = _get_compiled()
    in_maps = []
    for i in range(_NCORES):
        shard = xf[i * _S : (i + 1) * _S].transpose(1, 0, 2) * np.float32(_SCALE)
        sh = shard.astype(np.float16)
        sl = (shard - sh.astype(np.float32)).astype(np.float16)
        in_maps.append(
            {
                "xh": np.ascontiguousarray(sh),
                "xl": np.ascontiguousarray(sl),
                "w2h": W2h,
                "w2l": W2l,
                "w2": W2,
            }
        )
    res = run_bass_kernel_spmd(nc, in_maps, list(range(_NCORES)), trace=trace)

    quads = [[], [], [], []]  # LL, LH, HL, HH per-core chunks, each (S, 64, 64)
    for i in range(_NCORES):
        ot = res.results[i]["out_t"]  # (128, 96, 128) = [j(+64*qr), s, i(+64*qc)]
        quads[0].append(np.transpose(ot[0:64, :, 0:64], (1, 2, 0)))
        quads[1].append(np.transpose(ot[0:64, :, 64:128], (1, 2, 0)))
        quads[2].append(np.transpose(ot[64:128, :, 0:64], (1, 2, 0)))
        quads[3].append(np.transpose(ot[64:128, :, 64:128], (1, 2, 0)))

    B, C, H, W = x.shape
    out = tuple(
        np.ascontiguousarray(np.concatenate(q, axis=0)).reshape(B, C, H // 2, W // 2)
        for q in quads
    )
    return out, res.exec_time_ns


def kernel(x: np.ndarray, w_l: np.ndarray, w_h: np.ndarray):
    out, _ = run_on_hw(x, w_l, w_h, trace=False)
    return out
